# revision 3
# baseline (speedup 1.0000x reference)
"""Causal GQA self-attention (B=2, T=2048, C=2048, 16 heads / 4 KV groups,
head_size=128, RoPE) on 8 Trainium2 NeuronCores.

Sharding: tensor-parallel over the 4 KV groups x data-parallel over the 2
batch elements -> 8 cores, core = b*4 + g. Each core computes its group's
QKV projection, RoPE, causal SDPA for the group's 4 query heads, and the
partial output projection (w_proj input-dim shard). The proj partials are
reduced on the host (equivalent of the post-proj all-reduce).

v2 schedule: phase 1 is t-quarter-major with all six QKV accumulation
chains interleaved at contraction-chunk granularity, so the PE saturates
as soon as the first (w, x-quarter) DMA pair lands; x is DMA'd in
512-column quarters ordered to match. RoPE runs fully on DVE. Phase 2
packs pairs of score strips into one 2-bank PSUM tile so each exp
activation covers up to 1024 columns (halves ACT instruction overhead),
keeps attention row-sums on a PE ones-matmul, and double-buffers the
output projection through a PSUM pool shared with the row-sum tiles.
Output partials are written in bf16 (summed in fp32 on the host).
"""

import sys
import math

for _p in ("/opt/trn_rl_repo", "/root/.axon_site/_ro/trn_rl_repo"):
    if _p not in sys.path:
        sys.path.insert(0, _p)

import numpy as np
import ml_dtypes

import concourse.bass as bass  # noqa: F401  (registers engine classes)
import concourse.bacc as bacc
import concourse.tile as tile
from concourse import mybir
from concourse.bass_utils import run_bass_kernel_spmd
from concourse.masks import make_identity
from contextlib import ExitStack

BF16 = ml_dtypes.bfloat16
P = 128
T = 2048
C = 2048
NT = T // P        # 16 t-blocks
NCC = C // P       # 16 contraction chunks
NF = 6             # f-blocks per core: q0..q3, k, v
NQ = 4             # query heads per core
FQKV = NF * P      # 768
FY = NQ * P        # 512
SCALE = 1.0 / math.sqrt(P)
NEG = -1.0e30
FORDER = (4, 5, 0, 1, 2, 3)   # k, v, q0..q3

dt = mybir.dt
AF = mybir.ActivationFunctionType
ALU = mybir.AluOpType

TRACE = False
_CACHE = {}


def _build():
    nc = bacc.Bacc("TRN2", target_bir_lowering=False, debug=False, num_devices=8)
    xT_d = nc.dram_tensor("xT", [C, T], dt.bfloat16, kind="ExternalInput").ap()
    wqkT_d = nc.dram_tensor("wqkT", [C, FQKV], dt.bfloat16, kind="ExternalInput").ap()
    wpT_d = nc.dram_tensor("wpT", [FY, T], dt.bfloat16, kind="ExternalInput").ap()
    cosT_d = nc.dram_tensor("cosT", [P, T], dt.float32, kind="ExternalInput").ap()
    sinS_d = nc.dram_tensor("sinS", [P, T], dt.float32, kind="ExternalInput").ap()
    out_d = nc.dram_tensor("out", [T, C], dt.bfloat16, kind="ExternalOutput").ap()

    with tile.TileContext(nc) as tc, ExitStack() as ctx:
        const = ctx.enter_context(tc.tile_pool(name="const", bufs=1))
        identity = const.tile([P, P], dt.bfloat16, tag="id", name="identity")
        make_identity(nc, identity)
        ones_bf = const.tile([P, P], dt.bfloat16, tag="ones", name="ones_bf")
        nc.gpsimd.memset(ones_bf, 1.0)
        # causal mask for the diagonal 128x128 block of scores^T:
        # element (p=j, f=i): keep 0 where i - j >= 0, else -1e30
        maskf = const.tile([P, P], dt.float32, tag="mask", name="maskf")
        nc.gpsimd.memset(maskf, 0.0)
        nc.gpsimd.affine_select(
            out=maskf, in_=maskf, compare_op=ALU.is_ge, fill=NEG,
            base=0, pattern=[[1, P]], channel_multiplier=-1,
        )

        trig = ctx.enter_context(tc.tile_pool(name="trig", bufs=1))
        cosT = trig.tile([P, T], dt.float32, tag="cos", name="cosT")
        sinS = trig.tile([P, T], dt.float32, tag="sin", name="sinS")

        persist = ctx.enter_context(tc.tile_pool(name="persist", bufs=1))
        qrot = [persist.tile([P, T], dt.bfloat16, tag=f"q{h}", name=f"q{h}") for h in range(NQ)]
        krot = persist.tile([P, T], dt.bfloat16, tag="k", name="krot")
        vt = persist.tile([P, T], dt.bfloat16, tag="vt", name="vt")       # v t-major blocks
        y_sb = [persist.tile([P, T], dt.bfloat16, tag=f"y{h}", name=f"ysb{h}") for h in range(NQ)]
        wp_t = [persist.tile([P, T], dt.bfloat16, tag=f"wp{j}", name=f"wp{j}") for j in range(NQ)]

        # DMA order is the pipeline schedule: (w_ci, x_quarter3_ci) pairs feed
        # the first six interleaved QKV chains immediately; later x quarters
        # and trig quarters land just ahead of their consumers.
        xw_pool = ctx.enter_context(tc.tile_pool(name="xw", bufs=1))
        wt, xt = [], []
        for ci in range(NCC):
            tw = xw_pool.tile([P, FQKV], dt.bfloat16, tag=f"w{ci}", name=f"wt{ci}")
            wt.append(tw)
            tx = xw_pool.tile([P, T], dt.bfloat16, tag=f"x{ci}", name=f"xt{ci}")
            xt.append(tx)
        q3 = slice(3 * 512, 4 * 512)
        for ci in range(NCC):
            nc.sync.dma_start(wt[ci], wqkT_d[ci * P:(ci + 1) * P, :])
            nc.sync.dma_start(xt[ci][:, q3], xT_d[ci * P:(ci + 1) * P, q3])
        nc.sync.dma_start(cosT[:, q3], cosT_d[:, q3])
        nc.sync.dma_start(sinS[:, q3], sinS_d[:, q3])
        for t4 in (2, 1, 0):
            sq = slice(t4 * 512, (t4 + 1) * 512)
            for ci in range(NCC):
                nc.sync.dma_start(xt[ci][:, sq], xT_d[ci * P:(ci + 1) * P, sq])
            nc.sync.dma_start(cosT[:, sq], cosT_d[:, sq])
            nc.sync.dma_start(sinS[:, sq], sinS_d[:, sq])
        for j in range(NQ):
            nc.sync.dma_start(wp_t[j], wpT_d[j * P:(j + 1) * P, :])

        # ---------------- Phase 1: QKV^T = wqkT.T @ xT, fused RoPE ----------
        # Six chains (k, v, q0..q3) open per t-quarter, interleaved per
        # contraction chunk so the PE tracks DMA arrival exactly.
        with tc.tile_pool(name="rtmp", bufs=4) as rtmp, \
             tc.tile_pool(name="vstage", bufs=2) as vstage, \
             tc.tile_pool(name="qkvps", bufs=7, space="PSUM") as qkvps, \
             tc.tile_pool(name="vtps", bufs=1, space="PSUM") as vtps:
            for t4 in (3, 2, 1, 0):
                st = slice(t4 * 512, (t4 + 1) * 512)
                ps = {}
                for f in FORDER:
                    ps[f] = qkvps.tile([P, 512], dt.float32, tag="qkv", name=f"qkvps{f}")
                for ci in range(NCC):
                    for f in FORDER:
                        nc.tensor.matmul(
                            ps[f],
                            lhsT=wt[ci][:, f * P:(f + 1) * P],
                            rhs=xt[ci][:, st],
                            start=(ci == 0), stop=(ci == NCC - 1),
                        )
                for f in (4, 0, 1, 2, 3):
                    # RoPE (rotate-halves) in fp32, write bf16; all on DVE
                    dest = krot if f == 4 else qrot[f]
                    t1 = rtmp.tile([P, 512], dt.float32, tag="r1", name="ropet1")
                    nc.vector.tensor_mul(t1, ps[f], cosT[:, st])
                    t2 = rtmp.tile([P, 512], dt.float32, tag="r2", name="ropet2")
                    nc.vector.tensor_mul(t2[0:64, :], ps[f][64:128, :], sinS[0:64, st])
                    nc.vector.tensor_mul(t2[64:128, :], ps[f][0:64, :], sinS[64:128, st])
                    nc.vector.tensor_add(dest[:, st], t1, t2)
                # v^T -> v (t-major [j-part, d]) via PE transpose
                vr = vstage.tile([P, 512], dt.bfloat16, tag="vr", name="vraw")
                nc.scalar.activation(vr, ps[5], AF.Copy)
                for tb in range(4):
                    pst = vtps.tile([P, P], dt.bfloat16, tag="vtp", name="vtpst")
                    nc.tensor.transpose(pst, vr[:, tb * P:(tb + 1) * P], identity)
                    nc.vector.tensor_copy(vt[:, t4 * 512 + tb * P: t4 * 512 + (tb + 1) * P], pst)

        # ------------- Phase 2: attention + interleaved partial proj --------
        # Quarter-major over 512-wide i-windows (largest quarter first).
        # Score strips [j-part, i-free] are packed two-per-PSUM-tile
        # ([128,1024], 2 banks) so one ACT exp covers both. Row sums ride a
        # ones-matmul on the PE; proj chains share a double-buffered PSUM
        # pool with the row-sum tiles (they alternate in time).
        with tc.tile_pool(name="strip", bufs=6) as strip_pool, \
             tc.tile_pool(name="ssb", bufs=3) as ssb_pool, \
             tc.tile_pool(name="ostage", bufs=4) as ostage, \
             tc.tile_pool(name="scps", bufs=2, space="PSUM") as scps, \
             tc.tile_pool(name="ypsp", bufs=2, space="PSUM") as ypsp, \
             tc.tile_pool(name="mix", bufs=2, space="PSUM") as mixps:
            for q in (3, 2, 1, 0):
                q_lo = q * 512
                for h in range(NQ):
                    qT = qrot[h]
                    yps = ypsp.tile([P, 512], dt.float32, tag="y", name="ypst")
                    sps = mixps.tile([P, 512], dt.float32, tag="mx", name="spst")
                    njb = 4 * q + 4
                    strips = []
                    for jb in range(njb):
                        i_lo = max(jb * P, q_lo)
                        w = q_lo + 512 - i_lo
                        strips.append((jb, i_lo, w, 512 - w))
                    for pi in range(njb // 2):
                        pa, pb = strips[2 * pi], strips[2 * pi + 1]
                        duo = ((pa, 0), (pb, pa[2]))
                        sc = scps.tile([P, 1024], dt.float32, tag="sc", name="scpst")
                        stp = strip_pool.tile([P, 1024], dt.bfloat16, tag="strip", name="stript")
                        for (jb, i_lo, w, c0), off in duo:
                            nc.tensor.matmul(
                                sc[:, off:off + w],
                                lhsT=krot[:, jb * P:(jb + 1) * P],
                                rhs=qT[:, i_lo:i_lo + w],
                                start=True, stop=True,
                            )
                        for (jb, i_lo, w, c0), off in duo:
                            if jb >= 4 * q:  # diagonal block: apply causal mask
                                nc.vector.tensor_add(sc[:, off:off + P], sc[:, off:off + P], maskf)
                        wtot = pa[2] + pb[2]
                        nc.scalar.activation(stp[:, :wtot], sc[:, :wtot], AF.Exp, scale=SCALE)
                        for (jb, i_lo, w, c0), off in duo:
                            st_flag = (jb == 0)
                            sp_flag = (jb == njb - 1)
                            nc.tensor.matmul(
                                yps[:, c0:], lhsT=vt[:, jb * P:(jb + 1) * P],
                                rhs=stp[:, off:off + w],
                                start=st_flag, stop=sp_flag,
                            )
                            nc.tensor.matmul(
                                sps[:, c0:], lhsT=ones_bf, rhs=stp[:, off:off + w],
                                start=st_flag, stop=sp_flag,
                            )
                    # normalize: y * (1/rowsum) (sums broadcast on all partitions)
                    rcp = ssb_pool.tile([P, 512], dt.float32, tag="ssb", name="rcpt")
                    nc.vector.reciprocal_approx_fast(out=rcp, in_=sps)
                    nc.vector.tensor_mul(y_sb[h][:, q_lo:q_lo + 512], yps, rcp)
                # partial proj for this quarter's 4 t-blocks
                for tb in range(4 * q, 4 * q + 4):
                    t_sl = slice(tb * P, (tb + 1) * P)
                    for o4 in range(4):
                        o_sl = slice(o4 * 512, (o4 + 1) * 512)
                        pp = mixps.tile([P, 512], dt.float32, tag="mx", name="prpst")
                        for f4 in range(NQ):
                            nc.tensor.matmul(
                                pp, lhsT=y_sb[f4][:, t_sl], rhs=wp_t[f4][:, o_sl],
                                start=(f4 == 0), stop=(f4 == NQ - 1),
                            )
                        ot = ostage.tile([P, 512], dt.bfloat16, tag="o", name="otile")
                        if o4 % 2 == 0:
                            nc.scalar.activation(ot, pp, AF.Copy)
                        else:
                            nc.vector.tensor_copy(ot, pp)
                        nc.sync.dma_start(out_d[t_sl, o_sl], ot)

    nc.compile()
    return nc


def kernel(x, w_attn, w_proj, cos, sin):
    x = np.asarray(x, dtype=np.float32)
    w_attn = np.asarray(w_attn, dtype=np.float32)
    w_proj = np.asarray(w_proj, dtype=np.float32)
    cos = np.asarray(cos, dtype=np.float32)
    sin = np.asarray(sin, dtype=np.float32)

    if "nc" not in _CACHE:
        _CACHE["nc"] = _build()
    nc = _CACHE["nc"]

    cosT = np.ascontiguousarray(cos.T)                      # [128, T] f32
    sinT = np.ascontiguousarray(sin.T)
    sinS = sinT.copy()
    sinS[:64] = -sinS[:64]

    in_maps = []
    for core in range(8):
        b, g = core // 4, core % 4
        xT = np.ascontiguousarray(x[b].T).astype(BF16)                        # [C, T]
        wqkT = np.ascontiguousarray(w_attn[g * FQKV:(g + 1) * FQKV].T).astype(BF16)  # [C, 768]
        wpT = np.ascontiguousarray(w_proj[:, g * FY:(g + 1) * FY].T).astype(BF16)    # [512, T]
        in_maps.append({"xT": xT, "wqkT": wqkT, "wpT": wpT, "cosT": cosT, "sinS": sinS})

    res = run_bass_kernel_spmd(nc, in_maps, core_ids=list(range(8)), trace=TRACE)
    if TRACE:
        _CACHE["last_results"] = res

    out = np.zeros((2, T, C), dtype=np.float32)
    for core in range(8):
        b = core // 4
        out[b] += np.asarray(res.results[core]["out"], dtype=np.float32)
    return out


# revision 4
# speedup vs baseline: 1.0081x; 1.0081x over previous
"""Causal GQA self-attention (B=2, T=2048, C=2048, 16 heads / 4 KV groups,
head_size=128, RoPE) on 8 Trainium2 NeuronCores.

Sharding: tensor-parallel over the 4 KV groups x data-parallel over the 2
batch elements -> 8 cores, core = b*4 + g. Each core computes its group's
QKV projection, RoPE, causal SDPA for the group's 4 query heads, and the
partial output projection (w_proj input-dim shard). The proj partials are
reduced on the host (equivalent of the post-proj all-reduce).

v3 schedule:
- Phase 1 is t-quarter-major with all six QKV accumulation chains
  interleaved at contraction-chunk granularity, so the PE saturates as
  soon as the first (w, x-quarter-3) DMA pair lands. RoPE multiplies run
  on DVE, the final adds on GpSimd, and v-transpose staging copies on
  ACT, so no single engine queue serializes the phase transition.
- Phase 2 packs pairs of score strips into one 2-bank PSUM tile so each
  exp covers up to 1024 columns. Causal masking is applied AFTER exp by
  zeroing the upper triangle of the bf16 strip on GpSimd, and the masked
  (diagonal) pairs are computed FIRST in each chain so their extra hop
  hides under the off-diagonal strips. Row sums ride a ones-matmul into
  a dedicated double-buffered PSUM pool. The output projection runs as
  1024-wide double-chains in the same 2-bank pool as the score pairs
  (they alternate in time), with bf16 staging copies split between ACT
  and DVE. Output partials are bf16 (summed in fp32 on the host).
"""

import sys
import math

for _p in ("/opt/trn_rl_repo", "/root/.axon_site/_ro/trn_rl_repo"):
    if _p not in sys.path:
        sys.path.insert(0, _p)

import numpy as np
import ml_dtypes

import concourse.bass as bass  # noqa: F401  (registers engine classes)
import concourse.bacc as bacc
import concourse.tile as tile
from concourse import mybir
from concourse.bass_utils import run_bass_kernel_spmd
from concourse.masks import make_identity
from contextlib import ExitStack

BF16 = ml_dtypes.bfloat16
P = 128
T = 2048
C = 2048
NT = T // P        # 16 t-blocks
NCC = C // P       # 16 contraction chunks
NF = 6             # f-blocks per core: q0..q3, k, v
NQ = 4             # query heads per core
FQKV = NF * P      # 768
FY = NQ * P        # 512
SCALE = 1.0 / math.sqrt(P)
FORDER = (4, 5, 0, 1, 2, 3)   # k, v, q0..q3

dt = mybir.dt
AF = mybir.ActivationFunctionType
ALU = mybir.AluOpType

TRACE = False
_CACHE = {}


def _build():
    nc = bacc.Bacc("TRN2", target_bir_lowering=False, debug=False, num_devices=8)
    xT_d = nc.dram_tensor("xT", [C, T], dt.bfloat16, kind="ExternalInput").ap()
    wqkT_d = nc.dram_tensor("wqkT", [C, FQKV], dt.bfloat16, kind="ExternalInput").ap()
    wpT_d = nc.dram_tensor("wpT", [FY, T], dt.bfloat16, kind="ExternalInput").ap()
    cosT_d = nc.dram_tensor("cosT", [P, T], dt.float32, kind="ExternalInput").ap()
    sinS_d = nc.dram_tensor("sinS", [P, T], dt.float32, kind="ExternalInput").ap()
    out_d = nc.dram_tensor("out", [T, C], dt.bfloat16, kind="ExternalOutput").ap()

    with tile.TileContext(nc) as tc, ExitStack() as ctx:
        const = ctx.enter_context(tc.tile_pool(name="const", bufs=1))
        identity = const.tile([P, P], dt.bfloat16, tag="id", name="identity")
        make_identity(nc, identity)
        ones_bf = const.tile([P, P], dt.bfloat16, tag="ones", name="ones_bf")
        nc.gpsimd.memset(ones_bf, 1.0)

        trig = ctx.enter_context(tc.tile_pool(name="trig", bufs=1))
        cosT = trig.tile([P, T], dt.float32, tag="cos", name="cosT")
        sinS = trig.tile([P, T], dt.float32, tag="sin", name="sinS")

        persist = ctx.enter_context(tc.tile_pool(name="persist", bufs=1))
        qrot = [persist.tile([P, T], dt.bfloat16, tag=f"q{h}", name=f"q{h}") for h in range(NQ)]
        krot = persist.tile([P, T], dt.bfloat16, tag="k", name="krot")
        vt = persist.tile([P, T], dt.bfloat16, tag="vt", name="vt")       # v t-major blocks
        y_sb = [persist.tile([P, T], dt.bfloat16, tag=f"y{h}", name=f"ysb{h}") for h in range(NQ)]
        wp_t = [persist.tile([P, T], dt.bfloat16, tag=f"wp{j}", name=f"wp{j}") for j in range(NQ)]

        # DMA order is the pipeline schedule: (w_ci, x-quarter3_ci) pairs
        # feed the six interleaved QKV chains immediately; the remaining x
        # columns, trig, and proj weights follow.
        xw_pool = ctx.enter_context(tc.tile_pool(name="xw", bufs=1))
        wt, xt = [], []
        for ci in range(NCC):
            tw = xw_pool.tile([P, FQKV], dt.bfloat16, tag=f"w{ci}", name=f"wt{ci}")
            wt.append(tw)
            tx = xw_pool.tile([P, T], dt.bfloat16, tag=f"x{ci}", name=f"xt{ci}")
            xt.append(tx)
        q3 = slice(3 * 512, 4 * 512)
        rest = slice(0, 3 * 512)
        for ci in range(NCC):
            nc.sync.dma_start(wt[ci], wqkT_d[ci * P:(ci + 1) * P, :])
            nc.sync.dma_start(xt[ci][:, q3], xT_d[ci * P:(ci + 1) * P, q3])
        nc.sync.dma_start(cosT[:, q3], cosT_d[:, q3])
        nc.sync.dma_start(sinS[:, q3], sinS_d[:, q3])
        for ci in range(NCC):
            nc.sync.dma_start(xt[ci][:, rest], xT_d[ci * P:(ci + 1) * P, rest])
        nc.sync.dma_start(cosT[:, rest], cosT_d[:, rest])
        nc.sync.dma_start(sinS[:, rest], sinS_d[:, rest])
        for j in range(NQ):
            nc.sync.dma_start(wp_t[j], wpT_d[j * P:(j + 1) * P, :])

        # ---------------- Phase 1: QKV^T = wqkT.T @ xT, fused RoPE ----------
        with tc.tile_pool(name="rtmp", bufs=4) as rtmp, \
             tc.tile_pool(name="vstage", bufs=2) as vstage, \
             tc.tile_pool(name="qkvps", bufs=7, space="PSUM") as qkvps, \
             tc.tile_pool(name="vtps", bufs=1, space="PSUM") as vtps:
            for t4 in (3, 2, 1, 0):
                st = slice(t4 * 512, (t4 + 1) * 512)
                ps = {}
                for f in FORDER:
                    ps[f] = qkvps.tile([P, 512], dt.float32, tag="qkv", name=f"qkvps{f}")
                for ci in range(NCC):
                    for f in FORDER:
                        nc.tensor.matmul(
                            ps[f],
                            lhsT=wt[ci][:, f * P:(f + 1) * P],
                            rhs=xt[ci][:, st],
                            start=(ci == 0), stop=(ci == NCC - 1),
                        )
                for f in (4, 0, 1, 2, 3):
                    # RoPE (rotate-halves) in fp32, write bf16
                    dest = krot if f == 4 else qrot[f]
                    t1 = rtmp.tile([P, 512], dt.float32, tag="r1", name="ropet1")
                    nc.vector.tensor_mul(t1, ps[f], cosT[:, st])
                    t2 = rtmp.tile([P, 512], dt.float32, tag="r2", name="ropet2")
                    nc.vector.tensor_mul(t2[0:64, :], ps[f][64:128, :], sinS[0:64, st])
                    nc.vector.tensor_mul(t2[64:128, :], ps[f][0:64, :], sinS[64:128, st])
                    nc.gpsimd.tensor_add(dest[:, st], t1, t2)
                # v^T -> v (t-major [j-part, d]) via PE transpose
                vr = vstage.tile([P, 512], dt.bfloat16, tag="vr", name="vraw")
                nc.scalar.activation(vr, ps[5], AF.Copy)
                for tb in range(4):
                    pst = vtps.tile([P, P], dt.bfloat16, tag="vtp", name="vtpst")
                    nc.tensor.transpose(pst, vr[:, tb * P:(tb + 1) * P], identity)
                    nc.scalar.activation(vt[:, t4 * 512 + tb * P: t4 * 512 + (tb + 1) * P], pst, AF.Copy)

        # ------------- Phase 2: attention + interleaved partial proj --------
        with tc.tile_pool(name="strip", bufs=6) as strip_pool, \
             tc.tile_pool(name="ssb", bufs=3) as ssb_pool, \
             tc.tile_pool(name="ostage", bufs=4) as ostage, \
             tc.tile_pool(name="big", bufs=2, space="PSUM") as bigps, \
             tc.tile_pool(name="ypsp", bufs=2, space="PSUM") as ypsp, \
             tc.tile_pool(name="sums", bufs=2, space="PSUM") as sumps:
            for q in (3, 2, 1, 0):
                q_lo = q * 512
                for h in range(NQ):
                    qT = qrot[h]
                    yps = ypsp.tile([P, 512], dt.float32, tag="y", name="ypst")
                    sps = sumps.tile([P, 512], dt.float32, tag="s", name="spst")
                    njb = 4 * q + 4
                    # diagonal (masked) strips first so their post-exp mask
                    # hop hides under the off-diagonal work
                    order = list(range(4 * q, njb)) + list(range(0, 4 * q))
                    strips = []
                    for jb in order:
                        i_lo = max(jb * P, q_lo)
                        w = q_lo + 512 - i_lo
                        strips.append((jb, i_lo, w, 512 - w))
                    first_jb = strips[0][0]
                    last_jb = strips[-1][0]
                    for pi in range(njb // 2):
                        pa, pb = strips[2 * pi], strips[2 * pi + 1]
                        duo = ((pa, 0), (pb, pa[2]))
                        sc = bigps.tile([P, 1024], dt.float32, tag="big", name="scpst")
                        stp = strip_pool.tile([P, 1024], dt.bfloat16, tag="strip", name="stript")
                        for (jb, i_lo, w, c0), off in duo:
                            nc.tensor.matmul(
                                sc[:, off:off + w],
                                lhsT=krot[:, jb * P:(jb + 1) * P],
                                rhs=qT[:, i_lo:i_lo + w],
                                start=True, stop=True,
                            )
                        wtot = pa[2] + pb[2]
                        nc.scalar.activation(stp[:, :wtot], sc[:, :wtot], AF.Exp, scale=SCALE)
                        for (jb, i_lo, w, c0), off in duo:
                            if jb >= 4 * q:
                                # causal mask: zero upper triangle of the
                                # diagonal 128x128 block (post-exp, on GpSimd)
                                nc.gpsimd.affine_select(
                                    out=stp[:, off:off + P], in_=stp[:, off:off + P],
                                    compare_op=ALU.is_ge, fill=0.0,
                                    base=0, pattern=[[1, P]], channel_multiplier=-1,
                                )
                        for (jb, i_lo, w, c0), off in duo:
                            st_flag = (jb == first_jb)
                            sp_flag = (jb == last_jb)
                            nc.tensor.matmul(
                                yps[:, c0:], lhsT=vt[:, jb * P:(jb + 1) * P],
                                rhs=stp[:, off:off + w],
                                start=st_flag, stop=sp_flag,
                            )
                            nc.tensor.matmul(
                                sps[:, c0:], lhsT=ones_bf, rhs=stp[:, off:off + w],
                                start=st_flag, stop=sp_flag,
                            )
                    # normalize: y * (1/rowsum) (sums broadcast on all partitions)
                    rcp = ssb_pool.tile([P, 512], dt.float32, tag="ssb", name="rcpt")
                    nc.vector.reciprocal_approx_fast(out=rcp, in_=sps)
                    nc.vector.tensor_mul(y_sb[h][:, q_lo:q_lo + 512], yps, rcp)
                # partial proj for this quarter's 4 t-blocks, 1024-wide chains
                for tb in range(4 * q, 4 * q + 4):
                    t_sl = slice(tb * P, (tb + 1) * P)
                    for oh in range(2):
                        pp = bigps.tile([P, 1024], dt.float32, tag="big", name="prpst")
                        for f4 in range(NQ):
                            for o2 in range(2):
                                o_lo = oh * 1024 + o2 * 512
                                nc.tensor.matmul(
                                    pp[:, o2 * 512:(o2 + 1) * 512],
                                    lhsT=y_sb[f4][:, t_sl],
                                    rhs=wp_t[f4][:, o_lo:o_lo + 512],
                                    start=(f4 == 0), stop=(f4 == NQ - 1),
                                )
                        ot = ostage.tile([P, 1024], dt.bfloat16, tag="o", name="otile")
                        if oh == 0:
                            nc.scalar.activation(ot, pp, AF.Copy)
                        else:
                            nc.vector.tensor_copy(ot, pp)
                        nc.sync.dma_start(out_d[t_sl, oh * 1024:(oh + 1) * 1024], ot)

    nc.compile()
    return nc


def kernel(x, w_attn, w_proj, cos, sin):
    x = np.asarray(x, dtype=np.float32)
    w_attn = np.asarray(w_attn, dtype=np.float32)
    w_proj = np.asarray(w_proj, dtype=np.float32)
    cos = np.asarray(cos, dtype=np.float32)
    sin = np.asarray(sin, dtype=np.float32)

    if "nc" not in _CACHE:
        _CACHE["nc"] = _build()
    nc = _CACHE["nc"]

    cosT = np.ascontiguousarray(cos.T)                      # [128, T] f32
    sinT = np.ascontiguousarray(sin.T)
    sinS = sinT.copy()
    sinS[:64] = -sinS[:64]

    in_maps = []
    for core in range(8):
        b, g = core // 4, core % 4
        xT = np.ascontiguousarray(x[b].T).astype(BF16)                        # [C, T]
        wqkT = np.ascontiguousarray(w_attn[g * FQKV:(g + 1) * FQKV].T).astype(BF16)  # [C, 768]
        wpT = np.ascontiguousarray(w_proj[:, g * FY:(g + 1) * FY].T).astype(BF16)    # [512, T]
        in_maps.append({"xT": xT, "wqkT": wqkT, "wpT": wpT, "cosT": cosT, "sinS": sinS})

    res = run_bass_kernel_spmd(nc, in_maps, core_ids=list(range(8)), trace=TRACE)
    if TRACE:
        _CACHE["last_results"] = res

    out = np.zeros((2, T, C), dtype=np.float32)
    for core in range(8):
        b = core // 4
        out[b] += np.asarray(res.results[core]["out"], dtype=np.float32)
    return out


# revision 5
# speedup vs baseline: 1.1395x; 1.1303x over previous
"""Causal GQA self-attention (B=2, T=2048, C=2048, 16 heads / 4 KV groups,
head_size=128, RoPE) on 8 Trainium2 NeuronCores.

Sharding: tensor-parallel over the 4 KV groups x data-parallel over the 2
batch elements -> 8 cores, core = b*4 + g. Each core computes its group's
QKV projection, RoPE, causal SDPA for the group's 4 query heads, and the
partial output projection (w_proj input-dim shard). The proj partials are
reduced on the host (equivalent of the post-proj all-reduce).

v4 schedule:
- Phase 1 is t-quarter-major. The first quarter interleaves all six QKV
  accumulation chains at contraction-chunk granularity so the PE tracks
  the (w, x-quarter) DMA pairs as they land; the remaining quarters run
  chains sequentially (k, q0, v, q1, q2, q3) so RoPE drains spread
  evenly across DVE/GpSimd and k/q0 of the last quarter are rotated
  well before attention needs them.
- Phase 2 software-pipelines the whole attention stream with a one-pair
  lookahead: scores of pair n+1 are emitted before the PV/rowsum
  matmuls of pair n, so the exp (ACT) plus causal-zeroing (GpSimd)
  latency of each pair hides under the next pair's scores. Pairs of
  score strips share one 2-bank PSUM tile and a single exp; causal
  masking zeroes the upper triangle of the bf16 strip post-exp; masked
  pairs are computed first within each (head, quarter) chain. Row sums
  ride a ones-matmul into a double-buffered PSUM pool.
- The output projection runs at the end as 1024-wide double-chains
  rotating through the same 2-bank PSUM pool, with bf16 staging copies
  alternating between ACT and DVE. Output partials are bf16 (summed in
  fp32 on the host).
"""

import sys
import math

for _p in ("/opt/trn_rl_repo", "/root/.axon_site/_ro/trn_rl_repo"):
    if _p not in sys.path:
        sys.path.insert(0, _p)

import numpy as np
import ml_dtypes

import concourse.bass as bass  # noqa: F401  (registers engine classes)
import concourse.bacc as bacc
import concourse.tile as tile
from concourse import mybir
from concourse.bass_utils import run_bass_kernel_spmd
from concourse.masks import make_identity
from contextlib import ExitStack

BF16 = ml_dtypes.bfloat16
P = 128
T = 2048
C = 2048
NT = T // P        # 16 t-blocks
NCC = C // P       # 16 contraction chunks
NF = 6             # f-blocks per core: q0..q3, k, v
NQ = 4             # query heads per core
FQKV = NF * P      # 768
FY = NQ * P        # 512
SCALE = 1.0 / math.sqrt(P)
FORDER = (4, 5, 0, 1, 2, 3)       # k, v, q0..q3 (interleaved first quarter)
FSEQ = (4, 0, 5, 1, 2, 3)         # sequential quarters: k, q0, v, q1, q2, q3

dt = mybir.dt
AF = mybir.ActivationFunctionType
ALU = mybir.AluOpType

TRACE = False
_CACHE = {}


def _build():
    nc = bacc.Bacc("TRN2", target_bir_lowering=False, debug=False, num_devices=8)
    xT_d = nc.dram_tensor("xT", [C, T], dt.bfloat16, kind="ExternalInput").ap()
    wqkT_d = nc.dram_tensor("wqkT", [C, FQKV], dt.bfloat16, kind="ExternalInput").ap()
    wpT_d = nc.dram_tensor("wpT", [FY, T], dt.bfloat16, kind="ExternalInput").ap()
    cosT_d = nc.dram_tensor("cosT", [P, T], dt.float32, kind="ExternalInput").ap()
    sinS_d = nc.dram_tensor("sinS", [P, T], dt.float32, kind="ExternalInput").ap()
    out_d = nc.dram_tensor("out", [T, C], dt.bfloat16, kind="ExternalOutput").ap()

    with tile.TileContext(nc) as tc, ExitStack() as ctx:
        const = ctx.enter_context(tc.tile_pool(name="const", bufs=1))
        identity = const.tile([P, P], dt.bfloat16, tag="id", name="identity")
        make_identity(nc, identity)
        ones_bf = const.tile([P, P], dt.bfloat16, tag="ones", name="ones_bf")
        nc.gpsimd.memset(ones_bf, 1.0)

        trig = ctx.enter_context(tc.tile_pool(name="trig", bufs=1))
        cosT = trig.tile([P, T], dt.float32, tag="cos", name="cosT")
        sinS = trig.tile([P, T], dt.float32, tag="sin", name="sinS")

        persist = ctx.enter_context(tc.tile_pool(name="persist", bufs=1))
        qrot = [persist.tile([P, T], dt.bfloat16, tag=f"q{h}", name=f"q{h}") for h in range(NQ)]
        krot = persist.tile([P, T], dt.bfloat16, tag="k", name="krot")
        vt = persist.tile([P, T], dt.bfloat16, tag="vt", name="vt")       # v t-major blocks
        y_sb = [persist.tile([P, T], dt.bfloat16, tag=f"y{h}", name=f"ysb{h}") for h in range(NQ)]
        wp_t = [persist.tile([P, T], dt.bfloat16, tag=f"wp{j}", name=f"wp{j}") for j in range(NQ)]

        # DMA order is the pipeline schedule: (w_ci, x-quarter3_ci) pairs
        # feed the six interleaved QKV chains immediately; x quarter 2, the
        # remaining x columns, trig, and proj weights follow in consumption
        # order.
        xw_pool = ctx.enter_context(tc.tile_pool(name="xw", bufs=1))
        wt, xt = [], []
        for ci in range(NCC):
            tw = xw_pool.tile([P, FQKV], dt.bfloat16, tag=f"w{ci}", name=f"wt{ci}")
            wt.append(tw)
            tx = xw_pool.tile([P, T], dt.bfloat16, tag=f"x{ci}", name=f"xt{ci}")
            xt.append(tx)
        q3s = slice(3 * 512, 4 * 512)
        q2s = slice(2 * 512, 3 * 512)
        lo = slice(0, 2 * 512)
        for ci in range(NCC):
            nc.sync.dma_start(wt[ci], wqkT_d[ci * P:(ci + 1) * P, :])
            nc.sync.dma_start(xt[ci][:, q3s], xT_d[ci * P:(ci + 1) * P, q3s])
        nc.sync.dma_start(cosT[:, q3s], cosT_d[:, q3s])
        nc.sync.dma_start(sinS[:, q3s], sinS_d[:, q3s])
        for ci in range(NCC):
            nc.sync.dma_start(xt[ci][:, q2s], xT_d[ci * P:(ci + 1) * P, q2s])
        nc.sync.dma_start(cosT[:, q2s], cosT_d[:, q2s])
        nc.sync.dma_start(sinS[:, q2s], sinS_d[:, q2s])
        for ci in range(NCC):
            nc.sync.dma_start(xt[ci][:, lo], xT_d[ci * P:(ci + 1) * P, lo])
        nc.sync.dma_start(cosT[:, lo], cosT_d[:, lo])
        nc.sync.dma_start(sinS[:, lo], sinS_d[:, lo])
        for j in range(NQ):
            nc.sync.dma_start(wp_t[j], wpT_d[j * P:(j + 1) * P, :])

        # ---------------- Phase 1: QKV^T = wqkT.T @ xT, fused RoPE ----------
        with tc.tile_pool(name="rtmp", bufs=4) as rtmp, \
             tc.tile_pool(name="vstage", bufs=2) as vstage, \
             tc.tile_pool(name="qkvps", bufs=7, space="PSUM") as qkvps, \
             tc.tile_pool(name="vtps", bufs=1, space="PSUM") as vtps:

            def rope(f, pst, st):
                dest = krot if f == 4 else qrot[f]
                t1 = rtmp.tile([P, 512], dt.float32, tag="r1", name="ropet1")
                nc.vector.tensor_mul(t1, pst, cosT[:, st])
                t2 = rtmp.tile([P, 512], dt.float32, tag="r2", name="ropet2")
                nc.vector.tensor_mul(t2[0:64, :], pst[64:128, :], sinS[0:64, st])
                nc.vector.tensor_mul(t2[64:128, :], pst[0:64, :], sinS[64:128, st])
                nc.gpsimd.tensor_add(dest[:, st], t1, t2)

            def vtrans(pst, t4):
                vr = vstage.tile([P, 512], dt.bfloat16, tag="vr", name="vraw")
                nc.scalar.activation(vr, pst, AF.Copy)
                for tb in range(4):
                    pt = vtps.tile([P, P], dt.bfloat16, tag="vtp", name="vtpst")
                    nc.tensor.transpose(pt, vr[:, tb * P:(tb + 1) * P], identity)
                    nc.scalar.activation(
                        vt[:, t4 * 512 + tb * P: t4 * 512 + (tb + 1) * P], pt, AF.Copy)

            # quarter 3: six interleaved chains track DMA arrival
            st = q3s
            ps = {f: qkvps.tile([P, 512], dt.float32, tag="qkv", name=f"qkvps{f}")
                  for f in FORDER}
            for ci in range(NCC):
                for f in FORDER:
                    nc.tensor.matmul(
                        ps[f], lhsT=wt[ci][:, f * P:(f + 1) * P], rhs=xt[ci][:, st],
                        start=(ci == 0), stop=(ci == NCC - 1),
                    )
            for f in (4, 0, 1, 2, 3):
                rope(f, ps[f], st)
            vtrans(ps[5], 3)

            # quarters 2, 1, 0: sequential chains spread the RoPE drains
            for t4 in (2, 1, 0):
                st = slice(t4 * 512, (t4 + 1) * 512)
                for f in FSEQ:
                    pst = qkvps.tile([P, 512], dt.float32, tag="qkv", name=f"qkvps{f}")
                    for ci in range(NCC):
                        nc.tensor.matmul(
                            pst, lhsT=wt[ci][:, f * P:(f + 1) * P], rhs=xt[ci][:, st],
                            start=(ci == 0), stop=(ci == NCC - 1),
                        )
                    if f == 5:
                        vtrans(pst, t4)
                    else:
                        rope(f, pst, st)

        # ------------- Phase 2: attention (software-pipelined pairs) --------
        jobs = []
        for q in (3, 2, 1, 0):
            q_lo = q * 512
            njb = 4 * q + 4
            for h in range(NQ):
                order = list(range(4 * q, njb)) + list(range(0, 4 * q))
                strips = []
                for jb in order:
                    i_lo = max(jb * P, q_lo)
                    w = q_lo + 512 - i_lo
                    strips.append((jb, i_lo, w, 512 - w))
                for pi in range(njb // 2):
                    jobs.append(dict(
                        q=q, h=h, q_lo=q_lo,
                        pa=strips[2 * pi], pb=strips[2 * pi + 1],
                        first=(pi == 0), last=(pi == njb // 2 - 1),
                        first_jb=strips[0][0], last_jb=strips[-1][0],
                    ))

        with tc.tile_pool(name="strip", bufs=6) as strip_pool, \
             tc.tile_pool(name="ssb", bufs=3) as ssb_pool, \
             tc.tile_pool(name="ostage", bufs=4) as ostage, \
             tc.tile_pool(name="big", bufs=2, space="PSUM") as bigps, \
             tc.tile_pool(name="ypsp", bufs=2, space="PSUM") as ypsp, \
             tc.tile_pool(name="sums", bufs=2, space="PSUM") as sumps:
            acc = {}   # (q, h) -> (yps, sps)

            def emit_front(job):
                """scores matmuls + exp + causal zeroing for one pair"""
                qT = qrot[job['h']]
                sc = bigps.tile([P, 1024], dt.float32, tag="big", name="scpst")
                stp = strip_pool.tile([P, 1024], dt.bfloat16, tag="strip", name="stript")
                duo = ((job['pa'], 0), (job['pb'], job['pa'][2]))
                for (jb, i_lo, w, c0), off in duo:
                    nc.tensor.matmul(
                        sc[:, off:off + w],
                        lhsT=krot[:, jb * P:(jb + 1) * P],
                        rhs=qT[:, i_lo:i_lo + w],
                        start=True, stop=True,
                    )
                wtot = job['pa'][2] + job['pb'][2]
                nc.scalar.activation(stp[:, :wtot], sc[:, :wtot], AF.Exp, scale=SCALE)
                for (jb, i_lo, w, c0), off in duo:
                    if jb >= 4 * job['q']:
                        nc.gpsimd.affine_select(
                            out=stp[:, off:off + P], in_=stp[:, off:off + P],
                            compare_op=ALU.is_ge, fill=0.0,
                            base=0, pattern=[[1, P]], channel_multiplier=-1,
                        )
                return stp

            def emit_back(job, stp):
                """PV + rowsum accumulation for one pair; normalize at chain end"""
                key = (job['q'], job['h'])
                if job['first']:
                    acc[key] = (
                        ypsp.tile([P, 512], dt.float32, tag="y", name="ypst"),
                        sumps.tile([P, 512], dt.float32, tag="s", name="spst"),
                    )
                yps, sps = acc[key]
                duo = ((job['pa'], 0), (job['pb'], job['pa'][2]))
                for (jb, i_lo, w, c0), off in duo:
                    st_flag = (jb == job['first_jb'])
                    sp_flag = (jb == job['last_jb'])
                    nc.tensor.matmul(
                        yps[:, c0:], lhsT=vt[:, jb * P:(jb + 1) * P],
                        rhs=stp[:, off:off + w],
                        start=st_flag, stop=sp_flag,
                    )
                    nc.tensor.matmul(
                        sps[:, c0:], lhsT=ones_bf, rhs=stp[:, off:off + w],
                        start=st_flag, stop=sp_flag,
                    )
                if job['last']:
                    rcp = ssb_pool.tile([P, 512], dt.float32, tag="ssb", name="rcpt")
                    nc.vector.reciprocal_approx_fast(out=rcp, in_=sps)
                    nc.vector.tensor_mul(
                        y_sb[job['h']][:, job['q_lo']:job['q_lo'] + 512], yps, rcp)

            pending = None
            for job in jobs:
                stp = emit_front(job)
                if pending is not None:
                    emit_back(*pending)
                pending = (job, stp)
            emit_back(*pending)

            # ---------------- output projection, 1024-wide double-chains ----
            for tb in range(NT):
                t_sl = slice(tb * P, (tb + 1) * P)
                for oh in range(2):
                    pp = bigps.tile([P, 1024], dt.float32, tag="big", name="prpst")
                    for f4 in range(NQ):
                        for o2 in range(2):
                            o_lo = oh * 1024 + o2 * 512
                            nc.tensor.matmul(
                                pp[:, o2 * 512:(o2 + 1) * 512],
                                lhsT=y_sb[f4][:, t_sl],
                                rhs=wp_t[f4][:, o_lo:o_lo + 512],
                                start=(f4 == 0), stop=(f4 == NQ - 1),
                            )
                    ot = ostage.tile([P, 1024], dt.bfloat16, tag="o", name="otile")
                    if oh == 0:
                        nc.scalar.activation(ot, pp, AF.Copy)
                    else:
                        nc.vector.tensor_copy(ot, pp)
                    nc.sync.dma_start(out_d[t_sl, oh * 1024:(oh + 1) * 1024], ot)

    nc.compile()
    return nc


def kernel(x, w_attn, w_proj, cos, sin):
    x = np.asarray(x, dtype=np.float32)
    w_attn = np.asarray(w_attn, dtype=np.float32)
    w_proj = np.asarray(w_proj, dtype=np.float32)
    cos = np.asarray(cos, dtype=np.float32)
    sin = np.asarray(sin, dtype=np.float32)

    if "nc" not in _CACHE:
        _CACHE["nc"] = _build()
    nc = _CACHE["nc"]

    cosT = np.ascontiguousarray(cos.T)                      # [128, T] f32
    sinT = np.ascontiguousarray(sin.T)
    sinS = sinT.copy()
    sinS[:64] = -sinS[:64]

    in_maps = []
    for core in range(8):
        b, g = core // 4, core % 4
        xT = np.ascontiguousarray(x[b].T).astype(BF16)                        # [C, T]
        wqkT = np.ascontiguousarray(w_attn[g * FQKV:(g + 1) * FQKV].T).astype(BF16)  # [C, 768]
        wpT = np.ascontiguousarray(w_proj[:, g * FY:(g + 1) * FY].T).astype(BF16)    # [512, T]
        in_maps.append({"xT": xT, "wqkT": wqkT, "wpT": wpT, "cosT": cosT, "sinS": sinS})

    res = run_bass_kernel_spmd(nc, in_maps, core_ids=list(range(8)), trace=TRACE)
    if TRACE:
        _CACHE["last_results"] = res

    out = np.zeros((2, T, C), dtype=np.float32)
    for core in range(8):
        b = core // 4
        out[b] += np.asarray(res.results[core]["out"], dtype=np.float32)
    return out
